# revision 7
# baseline (speedup 1.0000x reference)
"""Causal average pooling (AverageContext) Trainium2 Bass kernel.

out[b, t, c] = mean_{s<=t} x[b, s, c]  for x [16, 4096, 128] fp32.
Data-parallel over batch: 2 batches per NeuronCore across 8 cores.

Layout (per batch): t = 32*p + j — partition p holds 32 consecutive rows
(16KB contiguous per partition; large-descriptor DMAs).

Everything stays in SBUF — GPSIMD has no PSUM port, and draining a
PSUM-resident scan through DVE/ACT was the old bottleneck. PSUM holds only
the [128, C] cross-partition carry B. Per batch:

  1. in-DMA in j-quarters, alternating SP/ACT (HWDGE), so the tree starts
     on the first quarter while the rest is in flight
  2. Pool adjacent-pairs tree L1/L2 -> P8[a] = sum of x over j in
     [8a, 8a+8)  (f32r tile, feeds PE directly)
  3. PE: B[p,c] = sum_{p'<p} sum_a P8[p',a,c] via 4 accumulating matmuls
     with an on-chip-generated strict-upper-tri stationary (no G reduction
     on the critical chain)
  4. Ctile[a-1] = sum_{j<8a} x = P8 prefixes (DVE for batch 0, Pool for
     batch 1 — overlaps the sibling batch's seed window)
  5. DVE seed: x3[:, a, 0, :] += B (+ Ctile[a-1] for a>=1), then in-place
     scan steps b=1..7: DVE on groups 0-1, Pool on groups 2-3 (two
     independent serial chains)
  6. scale x3 *= 1/(32p+j+1) in place, split DVE/Pool per j-quarter
     (stride-0 c-broadcast of the on-chip [128, 32] inv constant)
  7. out-DMA per j-quarter (SP/ACT) as soon as its region is scaled

Constants (tri matrix via iota+affine_select, inv via iota+reciprocal)
are generated on-chip; the DMA queues carry only input/output tiles.

A post-pass moves excess semaphore waits onto standalone
InstEventSemaphore instructions (walrus rejects >1 wait per instruction
and any wait on the f32r self-loading matmul's LW slot).
"""

import os
import sys

import numpy as np

for _p in (
    "/opt/trn_rl_repo",
    "/root/.axon_site",
    "/root/.axon_site/_ro/trn_rl_repo",
    "/root/.axon_site/_ro/pypackages",
):
    if os.path.isdir(_p) and _p not in sys.path:
        sys.path.append(_p)

import concourse.bass as bass  # noqa: E402
import concourse.mybir as mybir  # noqa: E402
import concourse.tile as tile  # noqa: E402

B, T, C = 16, 4096, 128
NCORES = 8
BPC = B // NCORES
P = 128
J = T // P  # 32
A, BB = 4, 8  # j = a*8 + b
H = J // 2  # 16

F32 = mybir.dt.float32
F32R = mybir.dt.float32r


def _legalize_sync_waits(nc):
    uid = [0]

    def mk_wait(engine, w):
        uid[0] += 1
        return mybir.InstEventSemaphore(
            name=f"I-waitfix-{uid[0]}",
            engine=engine,
            ins=[],
            outs=[],
            sync_info=mybir.SyncInfo(on_wait=[w], on_update=[]),
        )

    for f in nc.m.functions:
        for blk in f.blocks:
            out = []
            for inst in blk.instructions:
                si = inst.sync_info
                waits = list(si.on_wait) if si is not None and si.on_wait else []
                keep = 0 if type(inst).__name__ in ("InstMatmult", "InstLdweights") else 1
                if len(waits) > keep:
                    moved = waits[: len(waits) - keep] if keep else waits
                    kept = waits[len(waits) - keep :] if keep else []
                    for w in moved:
                        out.append(mk_wait(inst.engine, w))
                    inst.sync_info = mybir.SyncInfo(
                        on_wait=kept,
                        on_update=list(si.on_update) if si.on_update else [],
                    )
                out.append(inst)
            blk.instructions = out


def _bcast(ap_obj, dim_idx, count):
    dims = list(ap_obj.ap)
    dims.insert(dim_idx, [0, count])
    return bass.AP(tensor=ap_obj.tensor, offset=ap_obj.offset, ap=dims)


def _build_nc(legalize=True, reps=1, quarters=True, p8_on_dve=False,
              ct_on_pool=(1,), out_quarters=True, consts_onchip=True,
              hi_prio_scale=False, b1_l1_dve=0):
    from contextlib import ExitStack

    nc = bass.Bass()
    x_in = nc.declare_dram_parameter("x", [BPC, T, C], F32, isOutput=False)
    if not consts_onchip:
        lt_in = nc.declare_dram_parameter("lstrictT", [P, P], F32R, isOutput=False)
        inv_in = nc.declare_dram_parameter("invt", [P, J], F32, isOutput=False)
    y_out = nc.declare_dram_parameter("out", [BPC, T, C], F32, isOutput=True)

    with tile.TileContext(nc) as tc, ExitStack() as ctx:
        consts = ctx.enter_context(tc.tile_pool(name="consts", bufs=1))
        xp = ctx.enter_context(tc.tile_pool(name="xp", bufs=2))
        sc = ctx.enter_context(tc.tile_pool(name="sc", bufs=2))
        pp = ctx.enter_context(tc.tile_pool(name="pp", bufs=2, space="PSUM"))

        lt = consts.tile([P, P], F32R, tag="lt")
        invt = consts.tile([P, J], F32, tag="invt")
        consts_loaded = [False]
        if consts_onchip:
            # on-chip constant generation (no DMA): lt[p, f] = 1.0 iff f > p
            # (strict upper tri, f32r for PE), invt[p, j] = 1/(32p + j + 1)
            lt32 = consts.tile([P, P], F32, tag="lt32")
            nc.gpsimd.iota(
                lt32[:, :], pattern=[[0, P]], base=1, channel_multiplier=0,
                allow_small_or_imprecise_dtypes=True,
            )
            nc.gpsimd.affine_select(
                lt32[:, :], lt32[:, :], pattern=[[1, P]],
                compare_op=mybir.AluOpType.is_gt, fill=0.0,
                base=0, channel_multiplier=-1,
            )
            nc.vector.tensor_copy(lt[:, :], lt32[:, :])
            nc.gpsimd.iota(
                invt[:, :], pattern=[[1, J]], base=1, channel_multiplier=J,
                allow_small_or_imprecise_dtypes=True,
            )
            nc.vector.reciprocal(invt[:, :], invt[:, :])
            consts_loaded[0] = True

        for r in range(reps):
          for b in range(BPC):
            src_t = x_in if r == 0 else y_out
            x_dram = src_t[b].rearrange("(p j) c -> p j c", p=P)
            y_dram = y_out[b].rearrange("(p j) c -> p j c", p=P)

            x3 = xp.tile([P, J, C], F32, tag="x3")
            if quarters:
                Q = J // 4
                nc.sync.dma_start(out=x3[:, 0:Q, :], in_=x_dram[:, 0:Q, :])
                nc.scalar.dma_start(out=x3[:, Q : 2 * Q, :], in_=x_dram[:, Q : 2 * Q, :])
                nc.sync.dma_start(out=x3[:, 2 * Q : 3 * Q, :], in_=x_dram[:, 2 * Q : 3 * Q, :])
                nc.scalar.dma_start(out=x3[:, 3 * Q : J, :], in_=x_dram[:, 3 * Q : J, :])
            else:
                nc.sync.dma_start(out=x3[:, 0:H, :], in_=x_dram[:, 0:H, :])
                nc.scalar.dma_start(out=x3[:, H:J, :], in_=x_dram[:, H:J, :])
            if not consts_loaded[0]:
                # emitted after the first batch's inputs so inputs lead the
                # DMA queues; needed only by mm (~t+6us) and scale (~t+11us)
                nc.sync.dma_start(out=lt, in_=lt_in[:, :])
                nc.scalar.dma_start(out=invt, in_=inv_in[:, :])
                consts_loaded[0] = True

            # --- Pool half-trees: adjacent pairs -> P8 per half
            x2 = x3.rearrange("p (h two) c -> p h two c", two=2)
            l1 = sc.tile([P, 16, C], F32, tag="l1")
            l2 = sc.tile([P, 8, C], F32, tag="l2")
            p8 = sc.tile([P, A, C], F32R, tag="p8")
            l1r = l1.rearrange("p (h two) c -> p h two c", two=2)
            l2r = l2.rearrange("p (h two) c -> p h two c", two=2)
            p8_eng = nc.vector if p8_on_dve else nc.gpsimd
            # b1_l1_dve: number of j-halves of batch-1's L1 level to run on
            # DVE (idle pre-seed) instead of Pool (busy with batch-0's tree)
            n_dve_l1 = b1_l1_dve if b == 1 else 0
            if quarters:
                for h in range(2):
                    l1_eng = nc.vector if h < n_dve_l1 else nc.gpsimd
                    q0, q1 = slice(8 * h, 8 * h + 4), slice(8 * h + 4, 8 * h + 8)
                    s4, s2 = slice(4 * h, 4 * h + 4), slice(2 * h, 2 * h + 2)
                    l1_eng.tensor_add(l1[:, q0, :], x2[:, q0, 0, :], x2[:, q0, 1, :])
                    l1_eng.tensor_add(l1[:, q1, :], x2[:, q1, 0, :], x2[:, q1, 1, :])
                    nc.gpsimd.tensor_add(l2[:, s4, :], l1r[:, s4, 0, :], l1r[:, s4, 1, :])
                    p8_eng.tensor_add(p8[:, s2, :], l2r[:, s2, 0, :], l2r[:, s2, 1, :])
            else:
                for h in range(2):
                    s8, s4, s2 = slice(8 * h, 8 * h + 8), slice(4 * h, 4 * h + 4), slice(2 * h, 2 * h + 2)
                    nc.gpsimd.tensor_add(l1[:, s8, :], x2[:, s8, 0, :], x2[:, s8, 1, :])
                    nc.gpsimd.tensor_add(l2[:, s4, :], l1r[:, s4, 0, :], l1r[:, s4, 1, :])
                    p8_eng.tensor_add(p8[:, s2, :], l2r[:, s2, 0, :], l2r[:, s2, 1, :])

            # Ctile[a-1] = sum_{j < 8a} x  (a = 1..3)
            _ctp = (0, 1) if ct_on_pool is True else (ct_on_pool if isinstance(ct_on_pool, tuple) else ())
            ct_eng = nc.gpsimd if b in _ctp else nc.vector
            ct = sc.tile([P, A - 1, C], F32, tag="ct")
            ct_eng.tensor_copy(ct[:, 0, :], p8[:, 0, :])
            ct_eng.tensor_add(ct[:, 1, :], ct[:, 0, :], p8[:, 1, :])
            ct_eng.tensor_add(ct[:, 2, :], ct[:, 1, :], p8[:, 2, :])

            # --- PE: B[p, c] = sum_{p' < p} sum_a P8[p', a, c] via 4
            # accumulating matmuls (no serial G reduction on the chain)
            bp = pp.tile([P, C], F32, tag="bp")
            for a in range(A):
                nc.tensor.matmul(
                    bp, lt, p8[:, a, :], start=(a == 0), stop=(a == A - 1)
                )

            # --- DVE seed: x3[:, a, 0, :] += B (+ Ctile[a-1] for a >= 1)
            x4 = x3.rearrange("p (a bb) c -> p a bb c", bb=BB)
            nc.vector.tensor_add(x4[:, :, 0, :], x4[:, :, 0, :], _bcast(bp, 1, A))
            nc.vector.tensor_add(x4[:, 1:A, 0, :], x4[:, 1:A, 0, :], ct)

            # --- scan steps: DVE groups 0-1, Pool groups 2-3
            for bb in range(1, BB):
                nc.vector.tensor_add(
                    x4[:, 0:2, bb, :], x4[:, 0:2, bb - 1, :], x4[:, 0:2, bb, :]
                )
            for bb in range(1, BB):
                nc.gpsimd.tensor_add(
                    x4[:, 2:4, bb, :], x4[:, 2:4, bb - 1, :], x4[:, 2:4, bb, :]
                )

            # --- scale in place: DVE j[0:12), Pool j[12:16) then j[16:32)
            # (split so out-h1 only waits the small Pool op)
            from contextlib import nullcontext
            prio = tc.high_priority() if hi_prio_scale else nullcontext()
            if out_quarters:
              with prio:
                nc.vector.tensor_mul(
                    x3[:, 0:8, :], x3[:, 0:8, :], _bcast(invt[:, 0:8], 2, C)
                )
                nc.vector.tensor_mul(
                    x3[:, 8:12, :], x3[:, 8:12, :], _bcast(invt[:, 8:12], 2, C)
                )
                nc.gpsimd.tensor_mul(
                    x3[:, 12:16, :], x3[:, 12:16, :], _bcast(invt[:, 12:16], 2, C)
                )
                nc.gpsimd.tensor_mul(
                    x3[:, 16:24, :], x3[:, 16:24, :], _bcast(invt[:, 16:24], 2, C)
                )
                nc.gpsimd.tensor_mul(
                    x3[:, 24:32, :], x3[:, 24:32, :], _bcast(invt[:, 24:32], 2, C)
                )
                nc.scalar.dma_start(out=y_dram[:, 16:24, :], in_=x3[:, 16:24, :])
                nc.sync.dma_start(out=y_dram[:, 0:8, :], in_=x3[:, 0:8, :])
                nc.scalar.dma_start(out=y_dram[:, 24:32, :], in_=x3[:, 24:32, :])
                nc.sync.dma_start(out=y_dram[:, 8:16, :], in_=x3[:, 8:16, :])
            else:
                nc.vector.tensor_mul(
                    x3[:, 0:12, :], x3[:, 0:12, :], _bcast(invt[:, 0:12], 2, C)
                )
                nc.gpsimd.tensor_mul(
                    x3[:, 12:16, :], x3[:, 12:16, :], _bcast(invt[:, 12:16], 2, C)
                )
                nc.gpsimd.tensor_mul(
                    x3[:, 16:32, :], x3[:, 16:32, :], _bcast(invt[:, 16:32], 2, C)
                )
                nc.sync.dma_start(out=y_dram[:, 0:H, :], in_=x3[:, 0:H, :])
                nc.scalar.dma_start(out=y_dram[:, H:J, :], in_=x3[:, H:J, :])

    if legalize:
        _legalize_sync_waits(nc)
    return nc


def _make_consts():
    lstrictT = np.triu(np.ones((P, P), dtype=np.float32), 1)
    t_idx = np.arange(P)[:, None] * J + np.arange(J)[None, :]
    invt = (1.0 / (t_idx + 1.0)).astype(np.float32)
    return dict(lstrictT=lstrictT, invt=invt)


_NC = None


def _get_nc():
    global _NC
    if _NC is None:
        _NC = _build_nc()
    return _NC


def kernel(x: np.ndarray) -> np.ndarray:
    from concourse.bass_utils import run_bass_kernel_spmd

    assert x.shape == (B, T, C), x.shape
    x = np.asarray(x, dtype=np.float32)
    nc = _get_nc()
    # constants are generated on-chip (consts_onchip=True); only x is fed
    in_maps = [
        {"x": np.ascontiguousarray(x[i * BPC : (i + 1) * BPC])}
        for i in range(NCORES)
    ]
    res = run_bass_kernel_spmd(nc, in_maps, list(range(NCORES))).results
    return np.concatenate([res[i]["out"] for i in range(NCORES)], axis=0).astype(
        np.float32
    )


if __name__ == "__main__":
    x = np.random.randn(B, T, C).astype(np.float32)
    y = kernel(x)
    ref = np.cumsum(x, axis=1) / (np.arange(T) + 1.0)[None, :, None]
    err = np.abs(y - ref).max() / np.abs(ref).max()
    print("max abs-rel err:", err)


# revision 8
# speedup vs baseline: 2.8043x; 2.8043x over previous
"""Causal average pooling (AverageContext) Trainium2 Bass kernel.

out[b, t, c] = mean_{s<=t} x[b, s, c]  for x [16, 4096, 128] fp32.
Data-parallel over batch: 2 batches per NeuronCore across 8 cores.

Layout (per batch): t = 32*p + j — partition p holds 32 consecutive rows
(16KB contiguous per partition; large-descriptor DMAs).

Everything stays in SBUF — GPSIMD has no PSUM port, and draining a
PSUM-resident scan through DVE/ACT was the old bottleneck. PSUM holds only
the [128, C] cross-partition carry B. Per batch:

  1. in-DMA in j-quarters, alternating SP/ACT (HWDGE), so the tree starts
     on the first quarter while the rest is in flight
  2. Pool adjacent-pairs tree L1/L2 -> P8[a] = sum of x over j in
     [8a, 8a+8)  (f32r tile, feeds PE directly)
  3. PE: B[p,c] = sum_{p'<p} sum_a P8[p',a,c] via 4 accumulating matmuls
     with an on-chip-generated strict-upper-tri stationary (no G reduction
     on the critical chain)
  4. Ctile[a-1] = sum_{j<8a} x = P8 prefixes (DVE for batch 0, Pool for
     batch 1 — overlaps the sibling batch's seed window)
  5. DVE seed: x3[:, a, 0, :] += B (+ Ctile[a-1] for a>=1), then in-place
     scan steps b=1..7: DVE on groups 0-1, Pool on groups 2-3 (two
     independent serial chains)
  6. scale x3 *= 1/(32p+j+1) in place, split DVE/Pool per j-quarter
     (stride-0 c-broadcast of the on-chip [128, 32] inv constant)
  7. out-DMA per j-quarter (SP/ACT) as soon as its region is scaled

Constants (tri matrix via iota+affine_select, inv via iota+reciprocal)
are generated on-chip; the DMA queues carry only input/output tiles.

A post-pass moves excess semaphore waits onto standalone
InstEventSemaphore instructions (walrus rejects >1 wait per instruction
and any wait on the f32r self-loading matmul's LW slot).
"""

import os
import sys

import numpy as np

for _p in (
    "/opt/trn_rl_repo",
    "/root/.axon_site",
    "/root/.axon_site/_ro/trn_rl_repo",
    "/root/.axon_site/_ro/pypackages",
):
    if os.path.isdir(_p) and _p not in sys.path:
        sys.path.append(_p)

import concourse.bass as bass  # noqa: E402
import concourse.mybir as mybir  # noqa: E402
import concourse.tile as tile  # noqa: E402

B, T, C = 16, 4096, 128
NCORES = 8
BPC = B // NCORES
P = 128
J = T // P  # 32
A, BB = 4, 8  # j = a*8 + b
H = J // 2  # 16

F32 = mybir.dt.float32
F32R = mybir.dt.float32r


def _legalize_sync_waits(nc):
    uid = [0]

    def mk_wait(engine, w):
        uid[0] += 1
        return mybir.InstEventSemaphore(
            name=f"I-waitfix-{uid[0]}",
            engine=engine,
            ins=[],
            outs=[],
            sync_info=mybir.SyncInfo(on_wait=[w], on_update=[]),
        )

    for f in nc.m.functions:
        for blk in f.blocks:
            out = []
            for inst in blk.instructions:
                si = inst.sync_info
                waits = list(si.on_wait) if si is not None and si.on_wait else []
                keep = 0 if type(inst).__name__ in ("InstMatmult", "InstLdweights") else 1
                if len(waits) > keep:
                    moved = waits[: len(waits) - keep] if keep else waits
                    kept = waits[len(waits) - keep :] if keep else []
                    for w in moved:
                        out.append(mk_wait(inst.engine, w))
                    inst.sync_info = mybir.SyncInfo(
                        on_wait=kept,
                        on_update=list(si.on_update) if si.on_update else [],
                    )
                out.append(inst)
            blk.instructions = out


def _bcast(ap_obj, dim_idx, count):
    dims = list(ap_obj.ap)
    dims.insert(dim_idx, [0, count])
    return bass.AP(tensor=ap_obj.tensor, offset=ap_obj.offset, ap=dims)


def _build_nc(legalize=True, reps=1, quarters=False, p8_on_dve=False,
              ct_on_pool=(1,), out_quarters=False, consts_onchip=True,
              hi_prio_scale=False, b1_l1_dve=0):
    from contextlib import ExitStack

    nc = bass.Bass()
    x_in = nc.declare_dram_parameter("x", [BPC, T, C], F32, isOutput=False)
    if not consts_onchip:
        lt_in = nc.declare_dram_parameter("lstrictT", [P, P], F32R, isOutput=False)
        inv_in = nc.declare_dram_parameter("invt", [P, J], F32, isOutput=False)
    y_out = nc.declare_dram_parameter("out", [BPC, T, C], F32, isOutput=True)

    with tile.TileContext(nc) as tc, ExitStack() as ctx:
        consts = ctx.enter_context(tc.tile_pool(name="consts", bufs=1))
        xp = ctx.enter_context(tc.tile_pool(name="xp", bufs=2))
        sc = ctx.enter_context(tc.tile_pool(name="sc", bufs=2))
        pp = ctx.enter_context(tc.tile_pool(name="pp", bufs=2, space="PSUM"))

        lt = consts.tile([P, P], F32R, tag="lt")
        invt = consts.tile([P, J], F32, tag="invt")
        consts_loaded = [False]
        if consts_onchip:
            # on-chip constant generation (no DMA): lt[p, f] = 1.0 iff f > p
            # (strict upper tri, f32r for PE), invt[p, j] = 1/(32p + j + 1)
            lt32 = consts.tile([P, P], F32, tag="lt32")
            nc.gpsimd.iota(
                lt32[:, :], pattern=[[0, P]], base=1, channel_multiplier=0,
                allow_small_or_imprecise_dtypes=True,
            )
            nc.gpsimd.affine_select(
                lt32[:, :], lt32[:, :], pattern=[[1, P]],
                compare_op=mybir.AluOpType.is_gt, fill=0.0,
                base=0, channel_multiplier=-1,
            )
            nc.vector.tensor_copy(lt[:, :], lt32[:, :])
            nc.gpsimd.iota(
                invt[:, :], pattern=[[1, J]], base=1, channel_multiplier=J,
                allow_small_or_imprecise_dtypes=True,
            )
            nc.vector.reciprocal(invt[:, :], invt[:, :])
            consts_loaded[0] = True

        for r in range(reps):
          for b in range(BPC):
            src_t = x_in if r == 0 else y_out
            x_dram = src_t[b].rearrange("(p j) c -> p j c", p=P)
            y_dram = y_out[b].rearrange("(p j) c -> p j c", p=P)

            x3 = xp.tile([P, J, C], F32, tag="x3")
            if quarters:
                Q = J // 4
                nc.sync.dma_start(out=x3[:, 0:Q, :], in_=x_dram[:, 0:Q, :])
                nc.scalar.dma_start(out=x3[:, Q : 2 * Q, :], in_=x_dram[:, Q : 2 * Q, :])
                nc.sync.dma_start(out=x3[:, 2 * Q : 3 * Q, :], in_=x_dram[:, 2 * Q : 3 * Q, :])
                nc.scalar.dma_start(out=x3[:, 3 * Q : J, :], in_=x_dram[:, 3 * Q : J, :])
            else:
                nc.sync.dma_start(out=x3[:, 0:H, :], in_=x_dram[:, 0:H, :])
                nc.scalar.dma_start(out=x3[:, H:J, :], in_=x_dram[:, H:J, :])
            if not consts_loaded[0]:
                # emitted after the first batch's inputs so inputs lead the
                # DMA queues; needed only by mm (~t+6us) and scale (~t+11us)
                nc.sync.dma_start(out=lt, in_=lt_in[:, :])
                nc.scalar.dma_start(out=invt, in_=inv_in[:, :])
                consts_loaded[0] = True

            # --- Pool half-trees: adjacent pairs -> P8 per half
            x2 = x3.rearrange("p (h two) c -> p h two c", two=2)
            l1 = sc.tile([P, 16, C], F32, tag="l1")
            l2 = sc.tile([P, 8, C], F32, tag="l2")
            p8 = sc.tile([P, A, C], F32R, tag="p8")
            l1r = l1.rearrange("p (h two) c -> p h two c", two=2)
            l2r = l2.rearrange("p (h two) c -> p h two c", two=2)
            p8_eng = nc.vector if p8_on_dve else nc.gpsimd
            # b1_l1_dve: number of j-halves of batch-1's L1 level to run on
            # DVE (idle pre-seed) instead of Pool (busy with batch-0's tree)
            n_dve_l1 = b1_l1_dve if b == 1 else 0
            if quarters:
                for h in range(2):
                    l1_eng = nc.vector if h < n_dve_l1 else nc.gpsimd
                    q0, q1 = slice(8 * h, 8 * h + 4), slice(8 * h + 4, 8 * h + 8)
                    s4, s2 = slice(4 * h, 4 * h + 4), slice(2 * h, 2 * h + 2)
                    l1_eng.tensor_add(l1[:, q0, :], x2[:, q0, 0, :], x2[:, q0, 1, :])
                    l1_eng.tensor_add(l1[:, q1, :], x2[:, q1, 0, :], x2[:, q1, 1, :])
                    nc.gpsimd.tensor_add(l2[:, s4, :], l1r[:, s4, 0, :], l1r[:, s4, 1, :])
                    p8_eng.tensor_add(p8[:, s2, :], l2r[:, s2, 0, :], l2r[:, s2, 1, :])
            else:
                for h in range(2):
                    s8, s4, s2 = slice(8 * h, 8 * h + 8), slice(4 * h, 4 * h + 4), slice(2 * h, 2 * h + 2)
                    nc.gpsimd.tensor_add(l1[:, s8, :], x2[:, s8, 0, :], x2[:, s8, 1, :])
                    nc.gpsimd.tensor_add(l2[:, s4, :], l1r[:, s4, 0, :], l1r[:, s4, 1, :])
                    p8_eng.tensor_add(p8[:, s2, :], l2r[:, s2, 0, :], l2r[:, s2, 1, :])

            # Ctile[a-1] = sum_{j < 8a} x  (a = 1..3)
            _ctp = (0, 1) if ct_on_pool is True else (ct_on_pool if isinstance(ct_on_pool, tuple) else ())
            ct_eng = nc.gpsimd if b in _ctp else nc.vector
            ct = sc.tile([P, A - 1, C], F32, tag="ct")
            ct_eng.tensor_copy(ct[:, 0, :], p8[:, 0, :])
            ct_eng.tensor_add(ct[:, 1, :], ct[:, 0, :], p8[:, 1, :])
            ct_eng.tensor_add(ct[:, 2, :], ct[:, 1, :], p8[:, 2, :])

            # --- PE: B[p, c] = sum_{p' < p} sum_a P8[p', a, c] via 4
            # accumulating matmuls (no serial G reduction on the chain)
            bp = pp.tile([P, C], F32, tag="bp")
            for a in range(A):
                nc.tensor.matmul(
                    bp, lt, p8[:, a, :], start=(a == 0), stop=(a == A - 1)
                )

            # --- DVE seed: x3[:, a, 0, :] += B (+ Ctile[a-1] for a >= 1)
            x4 = x3.rearrange("p (a bb) c -> p a bb c", bb=BB)
            nc.vector.tensor_add(x4[:, :, 0, :], x4[:, :, 0, :], _bcast(bp, 1, A))
            nc.vector.tensor_add(x4[:, 1:A, 0, :], x4[:, 1:A, 0, :], ct)

            # --- scan steps: DVE groups 0-1, Pool groups 2-3
            for bb in range(1, BB):
                nc.vector.tensor_add(
                    x4[:, 0:2, bb, :], x4[:, 0:2, bb - 1, :], x4[:, 0:2, bb, :]
                )
            for bb in range(1, BB):
                nc.gpsimd.tensor_add(
                    x4[:, 2:4, bb, :], x4[:, 2:4, bb - 1, :], x4[:, 2:4, bb, :]
                )

            # --- scale in place: DVE j[0:12), Pool j[12:16) then j[16:32)
            # (split so out-h1 only waits the small Pool op)
            from contextlib import nullcontext
            prio = tc.high_priority() if hi_prio_scale else nullcontext()
            if out_quarters:
              with prio:
                nc.vector.tensor_mul(
                    x3[:, 0:8, :], x3[:, 0:8, :], _bcast(invt[:, 0:8], 2, C)
                )
                nc.vector.tensor_mul(
                    x3[:, 8:12, :], x3[:, 8:12, :], _bcast(invt[:, 8:12], 2, C)
                )
                nc.gpsimd.tensor_mul(
                    x3[:, 12:16, :], x3[:, 12:16, :], _bcast(invt[:, 12:16], 2, C)
                )
                nc.gpsimd.tensor_mul(
                    x3[:, 16:24, :], x3[:, 16:24, :], _bcast(invt[:, 16:24], 2, C)
                )
                nc.gpsimd.tensor_mul(
                    x3[:, 24:32, :], x3[:, 24:32, :], _bcast(invt[:, 24:32], 2, C)
                )
                nc.scalar.dma_start(out=y_dram[:, 16:24, :], in_=x3[:, 16:24, :])
                nc.sync.dma_start(out=y_dram[:, 0:8, :], in_=x3[:, 0:8, :])
                nc.scalar.dma_start(out=y_dram[:, 24:32, :], in_=x3[:, 24:32, :])
                nc.sync.dma_start(out=y_dram[:, 8:16, :], in_=x3[:, 8:16, :])
            else:
                nc.vector.tensor_mul(
                    x3[:, 0:12, :], x3[:, 0:12, :], _bcast(invt[:, 0:12], 2, C)
                )
                nc.gpsimd.tensor_mul(
                    x3[:, 12:16, :], x3[:, 12:16, :], _bcast(invt[:, 12:16], 2, C)
                )
                nc.gpsimd.tensor_mul(
                    x3[:, 16:32, :], x3[:, 16:32, :], _bcast(invt[:, 16:32], 2, C)
                )
                nc.sync.dma_start(out=y_dram[:, 0:H, :], in_=x3[:, 0:H, :])
                nc.scalar.dma_start(out=y_dram[:, H:J, :], in_=x3[:, H:J, :])

    if legalize:
        _legalize_sync_waits(nc)
    return nc


def _make_consts():
    lstrictT = np.triu(np.ones((P, P), dtype=np.float32), 1)
    t_idx = np.arange(P)[:, None] * J + np.arange(J)[None, :]
    invt = (1.0 / (t_idx + 1.0)).astype(np.float32)
    return dict(lstrictT=lstrictT, invt=invt)


_NC = None


def _get_nc():
    global _NC
    if _NC is None:
        _NC = _build_nc()
    return _NC


def kernel(x: np.ndarray) -> np.ndarray:
    from concourse.bass_utils import run_bass_kernel_spmd

    assert x.shape == (B, T, C), x.shape
    x = np.asarray(x, dtype=np.float32)
    nc = _get_nc()
    # constants are generated on-chip (consts_onchip=True); only x is fed
    in_maps = [
        {"x": np.ascontiguousarray(x[i * BPC : (i + 1) * BPC])}
        for i in range(NCORES)
    ]
    res = run_bass_kernel_spmd(nc, in_maps, list(range(NCORES))).results
    return np.concatenate([res[i]["out"] for i in range(NCORES)], axis=0).astype(
        np.float32
    )


if __name__ == "__main__":
    x = np.random.randn(B, T, C).astype(np.float32)
    y = kernel(x)
    ref = np.cumsum(x, axis=1) / (np.arange(T) + 1.0)[None, :, None]
    err = np.abs(y - ref).max() / np.abs(ref).max()
    print("max abs-rel err:", err)


# revision 13
# speedup vs baseline: 3.4072x; 1.2150x over previous
"""Causal average pooling (AverageContext) Trainium2 Bass kernel.

out[b, t, c] = mean_{s<=t} x[b, s, c]  for x [16, 4096, 128] fp32.
Data-parallel over batch: 2 batches per NeuronCore across 8 cores.

Layout (per batch): t = 32*p + j — partition p holds 32 consecutive rows
(16KB contiguous per partition; large-descriptor DMAs).

Everything stays in SBUF — GPSIMD has no PSUM port, and draining a
PSUM-resident scan through DVE/ACT was the old bottleneck. PSUM holds only
the [128, C] cross-partition carry B. Per batch:

  1. in-DMA in two 1MB j-halves, one on SP and one on ACT (HWDGE). Halves,
     not finer splits: 1MB transfers measured ~2.6x faster per iteration on
     hardware than 512KB quarters (fixed per-DMA cost dominates below ~1MB),
     even though the simulator's linear DMA model prefers quarters.
  2. Pool adjacent-pairs tree L1/L2 -> P8[a] = sum of x over j in
     [8a, 8a+8)  (f32r tile, feeds PE directly)
  3. PE: B[p,c] = sum_{p'<p} sum_a P8[p',a,c] via 4 accumulating matmuls
     with an on-chip-generated strict-upper-tri stationary (no G reduction
     on the critical chain)
  4. Ctile[a-1] = sum_{j<8a} x = P8 prefixes (DVE for batch 0, Pool for
     batch 1 — overlaps the sibling batch's seed window)
  5. DVE seed: x3[:, a, 0, :] += B (+ Ctile[a-1] for a>=1), then in-place
     scan steps b=1..7: DVE on groups 0-1, Pool on groups 2-3 (two
     independent serial chains)
  6. scale x3 *= 1/(32p+j+1) in place: DVE j[0:12), Pool j[12:16) and
     j[16:32) (stride-0 c-broadcast of the on-chip [128, 32] inv constant;
     the small Pool op decouples out-h1 from the big one)
  7. out-DMA per j-half (SP for j[0:16), ACT for j[16:32)) as soon as its
     region is scaled

Constants (tri matrix via iota+affine_select, inv via iota+reciprocal)
are generated on-chip; the DMA queues carry only input/output tiles.

A post-pass moves excess semaphore waits onto standalone
InstEventSemaphore instructions (walrus rejects >1 wait per instruction
and any wait on the f32r self-loading matmul's LW slot).
"""

import os
import sys

import numpy as np

for _p in (
    "/opt/trn_rl_repo",
    "/root/.axon_site",
    "/root/.axon_site/_ro/trn_rl_repo",
    "/root/.axon_site/_ro/pypackages",
):
    if os.path.isdir(_p) and _p not in sys.path:
        sys.path.append(_p)

import concourse.bass as bass  # noqa: E402
import concourse.mybir as mybir  # noqa: E402
import concourse.tile as tile  # noqa: E402

B, T, C = 16, 4096, 128
NCORES = 8
BPC = B // NCORES
P = 128
J = T // P  # 32
A, BB = 4, 8  # j = a*8 + b
H = J // 2  # 16

F32 = mybir.dt.float32
F32R = mybir.dt.float32r


def _legalize_sync_waits(nc):
    uid = [0]

    def mk_wait(engine, w):
        uid[0] += 1
        return mybir.InstEventSemaphore(
            name=f"I-waitfix-{uid[0]}",
            engine=engine,
            ins=[],
            outs=[],
            sync_info=mybir.SyncInfo(on_wait=[w], on_update=[]),
        )

    for f in nc.m.functions:
        for blk in f.blocks:
            out = []
            for inst in blk.instructions:
                si = inst.sync_info
                waits = list(si.on_wait) if si is not None and si.on_wait else []
                keep = 0 if type(inst).__name__ in ("InstMatmult", "InstLdweights") else 1
                if len(waits) > keep:
                    moved = waits[: len(waits) - keep] if keep else waits
                    kept = waits[len(waits) - keep :] if keep else []
                    for w in moved:
                        out.append(mk_wait(inst.engine, w))
                    inst.sync_info = mybir.SyncInfo(
                        on_wait=kept,
                        on_update=list(si.on_update) if si.on_update else [],
                    )
                out.append(inst)
            blk.instructions = out


def _bcast(ap_obj, dim_idx, count):
    dims = list(ap_obj.ap)
    dims.insert(dim_idx, [0, count])
    return bass.AP(tensor=ap_obj.tensor, offset=ap_obj.offset, ap=dims)


def _build_nc(legalize=True, reps=1, quarters=False, p8_on_dve=False,
              ct_on_pool=(1,), out_quarters=False, consts_onchip=True,
              hi_prio_scale=False, b1_l1_dve=0, full_dma=True):
    from contextlib import ExitStack

    nc = bass.Bass()
    x_in = nc.declare_dram_parameter("x", [BPC, T, C], F32, isOutput=False)
    if not consts_onchip:
        lt_in = nc.declare_dram_parameter("lstrictT", [P, P], F32R, isOutput=False)
        inv_in = nc.declare_dram_parameter("invt", [P, J], F32, isOutput=False)
    y_out = nc.declare_dram_parameter("out", [BPC, T, C], F32, isOutput=True)

    with tile.TileContext(nc) as tc, ExitStack() as ctx:
        consts = ctx.enter_context(tc.tile_pool(name="consts", bufs=1))
        xp = ctx.enter_context(tc.tile_pool(name="xp", bufs=2))
        sc = ctx.enter_context(tc.tile_pool(name="sc", bufs=2))
        pp = ctx.enter_context(tc.tile_pool(name="pp", bufs=2, space="PSUM"))

        lt = consts.tile([P, P], F32R, tag="lt")
        invt = consts.tile([P, J], F32, tag="invt")
        consts_loaded = [False]
        if consts_onchip:
            # on-chip constant generation (no DMA): lt[p, f] = 1.0 iff f > p
            # (strict upper tri, f32r for PE), invt[p, j] = 1/(32p + j + 1)
            lt32 = consts.tile([P, P], F32, tag="lt32")
            nc.gpsimd.iota(
                lt32[:, :], pattern=[[0, P]], base=1, channel_multiplier=0,
                allow_small_or_imprecise_dtypes=True,
            )
            nc.gpsimd.affine_select(
                lt32[:, :], lt32[:, :], pattern=[[1, P]],
                compare_op=mybir.AluOpType.is_gt, fill=0.0,
                base=0, channel_multiplier=-1,
            )
            nc.vector.tensor_copy(lt[:, :], lt32[:, :])
            nc.gpsimd.iota(
                invt[:, :], pattern=[[1, J]], base=1, channel_multiplier=J,
                allow_small_or_imprecise_dtypes=True,
            )
            nc.vector.reciprocal(invt[:, :], invt[:, :])
            consts_loaded[0] = True

        for r in range(reps):
          for b in range(BPC):
            src_t = x_in if r == 0 else y_out
            x_dram = src_t[b].rearrange("(p j) c -> p j c", p=P)
            y_dram = y_out[b].rearrange("(p j) c -> p j c", p=P)

            x3 = xp.tile([P, J, C], F32, tag="x3")
            if full_dma:
                # one 2MB transfer per batch; engine alternates per batch so
                # the two batches' inputs stream on both HWDGE rings
                (nc.sync if b == 0 else nc.scalar).dma_start(
                    out=x3, in_=x_dram
                )
            elif quarters:
                Q = J // 4
                nc.sync.dma_start(out=x3[:, 0:Q, :], in_=x_dram[:, 0:Q, :])
                nc.scalar.dma_start(out=x3[:, Q : 2 * Q, :], in_=x_dram[:, Q : 2 * Q, :])
                nc.sync.dma_start(out=x3[:, 2 * Q : 3 * Q, :], in_=x_dram[:, 2 * Q : 3 * Q, :])
                nc.scalar.dma_start(out=x3[:, 3 * Q : J, :], in_=x_dram[:, 3 * Q : J, :])
            else:
                nc.sync.dma_start(out=x3[:, 0:H, :], in_=x_dram[:, 0:H, :])
                nc.scalar.dma_start(out=x3[:, H:J, :], in_=x_dram[:, H:J, :])
            if not consts_loaded[0]:
                # emitted after the first batch's inputs so inputs lead the
                # DMA queues; needed only by mm (~t+6us) and scale (~t+11us)
                nc.sync.dma_start(out=lt, in_=lt_in[:, :])
                nc.scalar.dma_start(out=invt, in_=inv_in[:, :])
                consts_loaded[0] = True

            # --- Pool half-trees: adjacent pairs -> P8 per half
            x2 = x3.rearrange("p (h two) c -> p h two c", two=2)
            l1 = sc.tile([P, 16, C], F32, tag="l1")
            l2 = sc.tile([P, 8, C], F32, tag="l2")
            p8 = sc.tile([P, A, C], F32R, tag="p8")
            l1r = l1.rearrange("p (h two) c -> p h two c", two=2)
            l2r = l2.rearrange("p (h two) c -> p h two c", two=2)
            p8_eng = nc.vector if p8_on_dve else nc.gpsimd
            # b1_l1_dve: number of j-halves of batch-1's L1 level to run on
            # DVE (idle pre-seed) instead of Pool (busy with batch-0's tree)
            n_dve_l1 = b1_l1_dve if b == 1 else 0
            if quarters:
                for h in range(2):
                    l1_eng = nc.vector if h < n_dve_l1 else nc.gpsimd
                    q0, q1 = slice(8 * h, 8 * h + 4), slice(8 * h + 4, 8 * h + 8)
                    s4, s2 = slice(4 * h, 4 * h + 4), slice(2 * h, 2 * h + 2)
                    l1_eng.tensor_add(l1[:, q0, :], x2[:, q0, 0, :], x2[:, q0, 1, :])
                    l1_eng.tensor_add(l1[:, q1, :], x2[:, q1, 0, :], x2[:, q1, 1, :])
                    nc.gpsimd.tensor_add(l2[:, s4, :], l1r[:, s4, 0, :], l1r[:, s4, 1, :])
                    p8_eng.tensor_add(p8[:, s2, :], l2r[:, s2, 0, :], l2r[:, s2, 1, :])
            else:
                for h in range(2):
                    s8, s4, s2 = slice(8 * h, 8 * h + 8), slice(4 * h, 4 * h + 4), slice(2 * h, 2 * h + 2)
                    nc.gpsimd.tensor_add(l1[:, s8, :], x2[:, s8, 0, :], x2[:, s8, 1, :])
                    nc.gpsimd.tensor_add(l2[:, s4, :], l1r[:, s4, 0, :], l1r[:, s4, 1, :])
                    p8_eng.tensor_add(p8[:, s2, :], l2r[:, s2, 0, :], l2r[:, s2, 1, :])

            # Ctile[a-1] = sum_{j < 8a} x  (a = 1..3)
            _ctp = (0, 1) if ct_on_pool is True else (ct_on_pool if isinstance(ct_on_pool, tuple) else ())
            ct_eng = nc.gpsimd if b in _ctp else nc.vector
            ct = sc.tile([P, A - 1, C], F32, tag="ct")
            ct_eng.tensor_copy(ct[:, 0, :], p8[:, 0, :])
            ct_eng.tensor_add(ct[:, 1, :], ct[:, 0, :], p8[:, 1, :])
            ct_eng.tensor_add(ct[:, 2, :], ct[:, 1, :], p8[:, 2, :])

            # --- PE: B[p, c] = sum_{p' < p} sum_a P8[p', a, c] via 4
            # accumulating matmuls (no serial G reduction on the chain)
            bp = pp.tile([P, C], F32, tag="bp")
            for a in range(A):
                nc.tensor.matmul(
                    bp, lt, p8[:, a, :], start=(a == 0), stop=(a == A - 1)
                )

            # --- DVE seed: x3[:, a, 0, :] += B (+ Ctile[a-1] for a >= 1)
            x4 = x3.rearrange("p (a bb) c -> p a bb c", bb=BB)
            nc.vector.tensor_add(x4[:, :, 0, :], x4[:, :, 0, :], _bcast(bp, 1, A))
            nc.vector.tensor_add(x4[:, 1:A, 0, :], x4[:, 1:A, 0, :], ct)

            # --- scan steps: DVE groups 0-1, Pool groups 2-3
            for bb in range(1, BB):
                nc.vector.tensor_add(
                    x4[:, 0:2, bb, :], x4[:, 0:2, bb - 1, :], x4[:, 0:2, bb, :]
                )
            for bb in range(1, BB):
                nc.gpsimd.tensor_add(
                    x4[:, 2:4, bb, :], x4[:, 2:4, bb - 1, :], x4[:, 2:4, bb, :]
                )

            # --- scale in place: DVE j[0:12), Pool j[12:16) then j[16:32)
            # (split so out-h1 only waits the small Pool op)
            from contextlib import nullcontext
            prio = tc.high_priority() if hi_prio_scale else nullcontext()
            if out_quarters:
              with prio:
                nc.vector.tensor_mul(
                    x3[:, 0:8, :], x3[:, 0:8, :], _bcast(invt[:, 0:8], 2, C)
                )
                nc.vector.tensor_mul(
                    x3[:, 8:12, :], x3[:, 8:12, :], _bcast(invt[:, 8:12], 2, C)
                )
                nc.gpsimd.tensor_mul(
                    x3[:, 12:16, :], x3[:, 12:16, :], _bcast(invt[:, 12:16], 2, C)
                )
                nc.gpsimd.tensor_mul(
                    x3[:, 16:24, :], x3[:, 16:24, :], _bcast(invt[:, 16:24], 2, C)
                )
                nc.gpsimd.tensor_mul(
                    x3[:, 24:32, :], x3[:, 24:32, :], _bcast(invt[:, 24:32], 2, C)
                )
                nc.scalar.dma_start(out=y_dram[:, 16:24, :], in_=x3[:, 16:24, :])
                nc.sync.dma_start(out=y_dram[:, 0:8, :], in_=x3[:, 0:8, :])
                nc.scalar.dma_start(out=y_dram[:, 24:32, :], in_=x3[:, 24:32, :])
                nc.sync.dma_start(out=y_dram[:, 8:16, :], in_=x3[:, 8:16, :])
            else:
                nc.vector.tensor_mul(
                    x3[:, 0:12, :], x3[:, 0:12, :], _bcast(invt[:, 0:12], 2, C)
                )
                nc.gpsimd.tensor_mul(
                    x3[:, 12:16, :], x3[:, 12:16, :], _bcast(invt[:, 12:16], 2, C)
                )
                nc.gpsimd.tensor_mul(
                    x3[:, 16:32, :], x3[:, 16:32, :], _bcast(invt[:, 16:32], 2, C)
                )
                if full_dma:
                    # one 2MB out per batch on the opposite ring from its in
                    (nc.scalar if b == 0 else nc.sync).dma_start(
                        out=y_dram, in_=x3
                    )
                else:
                    nc.sync.dma_start(out=y_dram[:, 0:H, :], in_=x3[:, 0:H, :])
                    nc.scalar.dma_start(out=y_dram[:, H:J, :], in_=x3[:, H:J, :])

    if legalize:
        _legalize_sync_waits(nc)
    return nc


def _make_consts():
    lstrictT = np.triu(np.ones((P, P), dtype=np.float32), 1)
    t_idx = np.arange(P)[:, None] * J + np.arange(J)[None, :]
    invt = (1.0 / (t_idx + 1.0)).astype(np.float32)
    return dict(lstrictT=lstrictT, invt=invt)


_NC = None


def _get_nc():
    global _NC
    if _NC is None:
        _NC = _build_nc()
    return _NC


def kernel(x: np.ndarray) -> np.ndarray:
    from concourse.bass_utils import run_bass_kernel_spmd

    assert x.shape == (B, T, C), x.shape
    x = np.asarray(x, dtype=np.float32)
    nc = _get_nc()
    # constants are generated on-chip (consts_onchip=True); only x is fed
    in_maps = [
        {"x": np.ascontiguousarray(x[i * BPC : (i + 1) * BPC])}
        for i in range(NCORES)
    ]
    res = run_bass_kernel_spmd(nc, in_maps, list(range(NCORES))).results
    return np.concatenate([res[i]["out"] for i in range(NCORES)], axis=0).astype(
        np.float32
    )


if __name__ == "__main__":
    x = np.random.randn(B, T, C).astype(np.float32)
    y = kernel(x)
    ref = np.cumsum(x, axis=1) / (np.arange(T) + 1.0)[None, :, None]
    err = np.abs(y - ref).max() / np.abs(ref).max()
    print("max abs-rel err:", err)
